# revision 23
# baseline (speedup 1.0000x reference)
"""Trainium2 Bass kernel for 16-head causal attention (transposed-softmax variant).

Problem shapes: x [8, 1024, 1024]; W_K/W_Q/W_V [16, 64, 1024]; W_O [1024, 1024].
Sharding: pure data-parallel over batch (8 batch elements -> 8 cores), weights
replicated, no collectives.

Per-core pipeline (one batch element, seq=1024, d_embed=1024, 16 heads x 64):
  1. QKV projections as K_T/Q_T [heads*64, seq] and V [seq, heads*64], fp16
     operands, fp32 PSUM accumulation. W_Q is pre-scaled by 1/sqrt(d_head).
  2. Per head pair: scores S[c, C] for causal-allowed chunks, two heads
     concurrent in disjoint 64-row PE groups; triangular diag mask accumulated
     via identity matmul.  Rows i<4 put both heads side by side in one 2-bank
     PSUM tile; rows i>=4 use one 2-bank tile per head.
  3. Softmax over C without max-subtraction: exp per (row, head) on ScalarE
     with accum_out row sums collected into a per-pair [128,16] tile; one
     reciprocal + two broadcast tensor_tensor muls fold the normalization into
     V rows (V' = V/rowsum), applied lazily at the start of the next pair.
  4. zT[f, C] += V'^T E ascending in i (start flags at i=0 / i=4 per 512-chunk,
     stop at i=7), lagged a half-pair (self-hosted per-row for the last pair).
  5. Output projection split along the contraction: out_A (zT pairs 0-3) runs
     as dense filler inside pairs 5-7 and parks in SBUF fp16; out_B (pairs 4-7)
     runs in the tail and is combined via tensor_tensor add.

Scheduling: the scalar engine (exp chain) paces each pair at ~20us while the
PE's own attention work is ~9us, so dense projection groups are drip-fed as
fillers into every pair with just-in-time deadlines.  This keeps PE activity
density high everywhere so the HAM clock gate stays at K=8/8 (a previous
version ran the whole second half at 1.2 GHz because fillers were exhausted
after pair 3).
"""

import numpy as np

S, E, A, H, B = 1024, 1024, 16, 64, 8
P = 128          # partitions
NEG = -30000.0   # additive mask value (fp16-safe; exp -> 0 in fp32)

_cache = {}


def _off(i):
    """Compact E-buffer offset of row-tile i (valid width of row i is (i+1)*P)."""
    return P * i * (i + 1) // 2


EW = _off(8)     # 4608 columns total


def _build_nc():
    import concourse.bass as bass
    import concourse.mybir as mybir
    from concourse.tile import TileContext

    f16 = mybir.dt.float16
    f32 = mybir.dt.float32
    Exp = mybir.ActivationFunctionType.Exp

    nc = bass.Bass()
    xt_d = nc.dram_tensor("xt", [E, S], f16, kind="ExternalInput")        # x[b].T
    wkqv_d = nc.dram_tensor("wkqv", [E, 3 * A * H], f16, kind="ExternalInput")
    wo_d = nc.dram_tensor("wo", [A * H, E], f16, kind="ExternalInput")    # W_O.T
    msk_d = nc.dram_tensor("msk", [P, P], f16, kind="ExternalInput")
    id_d = nc.dram_tensor("ident", [P, P], f16, kind="ExternalInput")
    out_d = nc.dram_tensor("out", [S, E], f16, kind="ExternalOutput")

    ET = E // P       # 8 e-tiles
    CT = S // P       # 8 c-tiles
    NC = S // 512     # 2 512-chunks

    with TileContext(nc) as tc:
        with (
            tc.tile_pool(name="inp", bufs=1) as inp,
            tc.tile_pool(name="kqv", bufs=1) as kqv,
            tc.tile_pool(name="epool", bufs=4) as epool,
            tc.tile_pool(name="stats", bufs=4) as stats,
            tc.tile_pool(name="outp", bufs=3) as outp,
            tc.tile_pool(name="pss", bufs=2, space="PSUM") as pss,   # 4 banks
            tc.tile_pool(name="psq", bufs=2, space="PSUM") as psq,   # 2 banks
            tc.tile_pool(name="pza", bufs=2, space="PSUM") as pza,   # 2 banks
        ):
            # ---- SBUF destinations ----
            xT = inp.tile([P, ET, S], f16, tag="xT")
            wkqv = inp.tile([P, ET, 3 * A * H], f16, tag="wkqv")
            wo = inp.tile([P, ET, E], f16, tag="wo")
            msk = inp.tile([P, P], f16, tag="msk")
            ident = inp.tile([P, P], f16, tag="ident")
            warm = inp.tile([P, 512], f16, tag="warm")

            nc.any.memset(warm[:], 0.125)

            # ---- loads, ordered by first use.  The sync engine needs ~650ns
            # per DMA_DIRECT2D issue, so the critical prefix (x first half,
            # K/Q weights of pairs 0-3) is issued from three engines in
            # parallel; the rest dribbles out on sync in deadline order. ----
            nc.sync.dma_start(ident[:], id_d[:])
            nc.sync.dma_start(msk[:], msk_d[:])
            for t in range(ET):                       # x first 512 c's
                nc.sync.dma_start(xT[:, t, 0:512], xt_d[t * P:(t + 1) * P, 0:512])
            for t in range(ET):                       # K weights, pairs 0-3
                nc.scalar.dma_start(wkqv[:, t, 0:512],
                                    wkqv_d[t * P:(t + 1) * P, 0:512])
            for t in range(ET):                       # Q weights, pairs 0-3
                eng = nc.scalar if t < 4 else nc.sync
                eng.dma_start(wkqv[:, t, A * H:A * H + 512],
                              wkqv_d[t * P:(t + 1) * P, A * H:A * H + 512])
            for t in range(ET):                       # x second half
                nc.sync.dma_start(xT[:, t, 512:S], xt_d[t * P:(t + 1) * P, 512:S])
            for t in range(ET):                       # V weights, f 0:512
                nc.sync.dma_start(wkqv[:, t, 2 * A * H:2 * A * H + 512],
                                  wkqv_d[t * P:(t + 1) * P, 2 * A * H:2 * A * H + 512])
            for t in range(ET):                       # K weights, pairs 4-7
                nc.sync.dma_start(wkqv[:, t, 512:A * H],
                                  wkqv_d[t * P:(t + 1) * P, 512:A * H])
            for t in range(ET):                       # Q weights, pairs 4-7
                nc.sync.dma_start(wkqv[:, t, A * H + 512:2 * A * H],
                                  wkqv_d[t * P:(t + 1) * P, A * H + 512:2 * A * H])
            for t in range(ET):                       # V weights, f 512:1024
                nc.sync.dma_start(wkqv[:, t, 2 * A * H + 512:3 * A * H],
                                  wkqv_d[t * P:(t + 1) * P, 2 * A * H + 512:3 * A * H])
            for t in range(ET):                       # output projection weights
                nc.sync.dma_start(wo[:, t, :], wo_d[t * P:(t + 1) * P, :])

            K_T = kqv.tile([P, A // 2, S], f16, tag="K_T")   # pair-stacked [2h, c]
            Q_T = kqv.tile([P, A // 2, S], f16, tag="Q_T")
            V = kqv.tile([P, CT, A * H], f16, tag="V")       # [c, f]
            zT = kqv.tile([P, A // 2, S], f16, tag="zT")     # pair-stacked [f, C]
            oacc = kqv.tile([P, CT, E], f16, tag="oacc")     # out_A parking

            # ---- PE warm-up: ride out the HAM throttle during the DMA wait ----
            wps = psq.tile([P, 512], f32, tag="psq", name="warm")
            for w in range(28):
                nc.tensor.matmul(wps[:], ident[:], warm[:],
                                 start=(w == 0), stop=(w == 27),
                                 skip_group_check=True)
            wsb = stats.tile([P, 1], f32, tag="wsink", name="warmsink")
            nc.vector.reduce_max(wsb[:], wps[:, :P], axis=mybir.AxisListType.X)

            # ---- dense projection groups (the filler pool) ----
            def kq_span(p, mat, lo, hi):
                dst = K_T if mat == 0 else Q_T
                w = hi - lo
                ps = psq.tile([P, 512], f32, tag="psq", name=f"q{p}{mat}{lo}")
                for et in range(ET):
                    nc.tensor.matmul(
                        ps[:, 0:w],
                        wkqv[:, et, mat * A * H + p * P: mat * A * H + (p + 1) * P],
                        xT[:, et, lo:hi],
                        start=(et == 0), stop=(et == ET - 1),
                    )
                nc.vector.tensor_copy(out=dst[:, p, lo:hi], in_=ps[:, 0:w])

            def kq_group(p, mat, cc):
                kq_span(p, mat, cc * 512, (cc + 1) * 512)

            def v_group(fc, i):
                ps = psq.tile([P, 512], f32, tag="psq", name=f"v{fc}{i}")
                for et in range(ET):
                    nc.tensor.matmul(
                        ps[:],
                        xT[:, et, i * P:(i + 1) * P],
                        wkqv[:, et, 2 * A * H + fc * 512: 2 * A * H + (fc + 1) * 512],
                        start=(et == 0), stop=(et == ET - 1),
                    )
                nc.vector.tensor_copy(out=V[:, i, fc * 512:(fc + 1) * 512], in_=ps[:])

            def oa_group(m, n_):
                """out_A: first-half contraction (zT pairs 0-3), parked fp16."""
                ps = psq.tile([P, 512], f32, tag="psq", name=f"oa{m}{n_}")
                for p2 in range(4):
                    nc.tensor.matmul(
                        ps[:],
                        zT[:, p2, m * P:(m + 1) * P],
                        wo[:, p2, n_ * 512:(n_ + 1) * 512],
                        start=(p2 == 0), stop=(p2 == 3),
                    )
                nc.vector.tensor_copy(out=oacc[:, m, n_ * 512:(n_ + 1) * 512], in_=ps[:])

            # ---- attention ----
            def attn_rows(p, pump, row_done=None):
                """Scores+exp rows of pair p.  pump(i) interleaves filler work
                after row i's exp chain; row_done(i, ...) optionally self-hosts
                AV steps (pair 7)."""
                heads = [(2 * p, 0), (2 * p + 1, H)]
                Ets = [epool.tile([P, EW], f16, tag="E", name=f"E{k}_{p}")
                       for k in range(2)]
                scat = stats.tile([P, 16], f32, tag="ssum", name=f"sc{p}")
                for i in range(CT):
                    vw = (i + 1) * P          # causally-valid row width
                    if i < 4:   # both heads side by side in one 2-bank tile
                        row = pss.tile([P, 1024], f32, tag="srow", name=f"r_{p}_{i}")
                        views = [row[:, 0:vw], row[:, 512:512 + vw]]
                        dviews = [row[:, i * P:(i + 1) * P],
                                  row[:, 512 + i * P:512 + i * P + P]]
                        n_i = 1
                    else:
                        rows = [pss.tile([P, 1024], f32, tag="srow",
                                         name=f"r{k}_{p}_{i}")
                                for k in range(2)]
                        views = [rows[0][:, 0:vw], rows[1][:, 0:vw]]
                        dviews = [rows[0][:, i * P:(i + 1) * P],
                                  rows[1][:, i * P:(i + 1) * P]]
                        n_i = 2
                    for j in range(n_i):
                        diag = j == n_i - 1
                        ntrim = vw - j * 512 if diag else 512
                        for k, (a, off) in enumerate(heads):
                            nc.tensor.matmul(
                                views[k][:, j * 512:j * 512 + ntrim],
                                K_T[off:off + H, p, i * P:(i + 1) * P],
                                Q_T[off:off + H, p, j * 512:j * 512 + ntrim],
                                start=True, stop=True,
                                skip_group_check=True,
                            )
                    # causal mask of the diagonal 128-block, added on the DVE
                    # (the PE is the bottleneck engine; the DVE has slack)
                    for k in range(2):
                        nc.vector.tensor_tensor(
                            dviews[k], dviews[k], msk[:], mybir.AluOpType.add,
                        )
                    for k, (a, off) in enumerate(heads):
                        nc.scalar.activation(
                            Ets[k][:, _off(i):_off(i) + vw], views[k], Exp,
                            accum_out=scat[:, k * 8 + i:k * 8 + i + 1],
                        )
                    pump(i)
                    if row_done is not None:
                        row_done(i, heads, Ets, scat)
                return heads, Ets, scat

            def av_step(p, heads, Ets, state, i):
                """One ascending AV accumulation step (both 512-chunks)."""
                if i == 0:
                    state[0] = pza.tile([P, 512], f32, tag="za", name=f"za_{p}_0")
                if i == 4:
                    state[1] = pza.tile([P, 512], f32, tag="za", name=f"za_{p}_1")
                for j in range(NC):
                    if i < 4 * j:
                        continue
                    ntrim = min(512, (i + 1) * P - j * 512)
                    for k, (a, off) in enumerate(heads):
                        nc.tensor.matmul(
                            state[j][off:off + H, :ntrim],
                            V[:, i, a * H:(a + 1) * H],
                            Ets[k][:, _off(i) + j * 512:_off(i) + j * 512 + ntrim],
                            start=(i == 4 * j), stop=(i == CT - 1),
                            skip_group_check=True,
                        )

            def scale_all(p, heads, scat):
                """Fold 1/rowsum into V rows of pair p (all 8 row-tiles)."""
                rcp = stats.tile([P, 16], f32, tag="rcp", name=f"rc{p}")
                nc.vector.reciprocal(rcp[:], scat[:])
                for k, (a, off) in enumerate(heads):
                    vs = V[:, :, a * H:(a + 1) * H]
                    nc.vector.tensor_tensor(
                        vs, vs,
                        rcp[:, k * 8:(k + 1) * 8, None].to_broadcast([P, CT, H]),
                        mybir.AluOpType.mult,
                    )

            def av_closures(p, heads, Ets, scat):
                """Lazy normalize + AV of pair p, interleaved into pair p+1.
                Pair 0's chain is pushed to row 3+ so its V groups (gated on
                the V-weight DMA) get breathing room."""
                off = 3 if p == 0 else 0
                state = {}
                cs = [(off, lambda: scale_all(p, heads, scat))]
                for i in range(CT):
                    cs.append((min(7, off + (i + 1) // 2),
                               lambda i=i: av_step(p, heads, Ets, state, i)))
                cs.append((min(7, off + 4), lambda: nc.vector.tensor_copy(
                    out=zT[:, p, 0:512], in_=state[0][:])))
                cs.append((min(7, off + 5), lambda: nc.vector.tensor_copy(
                    out=zT[:, p, 512:1024], in_=state[1][:])))
                return cs

            # ---- filler schedule -------------------------------------------
            # Queue of (deadline (pair,row), closure); before row r of pair p
            # every filler with deadline <= (p, r) is forced out.
            fillers = []

            def add(dl, cl):
                fillers.append((dl, cl))

            # pair 0/1 KQ halves not covered by the (minimal) pre-phase
            add((0, 0), lambda: kq_group(1, 0, 0))
            add((0, 1), lambda: kq_group(1, 1, 0))
            add((0, 2), lambda: kq_group(0, 0, 1))
            add((0, 3), lambda: kq_group(0, 1, 1))
            add((0, 5), lambda: kq_group(1, 0, 1))
            add((0, 6), lambda: kq_group(1, 1, 1))
            for p in range(2, 8):
                add((p - 2, 4), lambda p=p: kq_group(p, 0, 0))
                add((p - 2, 6), lambda p=p: kq_group(p, 1, 0))
                add((p - 1, 1), lambda p=p: kq_group(p, 0, 1))
                add((p - 1, 2), lambda p=p: kq_group(p, 1, 1))
            # V(fc0, i) needed by scale_all(0) at pair-1 row 3;
            # V(fc1, i) by scale_all(4) at pair-5 row 0.
            v0_dl = [(0, 4), (0, 6), (0, 7), (1, 0), (1, 0), (1, 1), (1, 2), (1, 3)]
            for i in range(CT):
                add(v0_dl[i], lambda i=i: v_group(0, i))
            v1_dl = [(2, 1), (2, 3), (2, 5), (3, 1), (3, 3), (4, 1), (4, 3), (4, 5)]
            for i in range(CT):
                add(v1_dl[i], lambda i=i: v_group(1, i))
            # out_A(m, n) after AV(3) copies (pair-4 row 5); spread pairs 4-7.
            oa_dl = [(4, 6), (4, 7), (5, 1), (5, 3), (5, 5), (5, 7),
                     (6, 1), (6, 2), (6, 3), (6, 4), (6, 5), (6, 6),
                     (7, 1), (7, 3), (7, 5), (7, 6)]
            for m in range(CT):
                for n_ in range(NC):
                    add(oa_dl[2 * m + n_], lambda m=m, n_=n_: oa_group(m, n_))

            fillers.sort(key=lambda x: x[0])
            fidx = [0]
            av_queue = []   # (deadline_row, closure) of the previous pair's AV

            def pump_factory(p):
                def pump(i):
                    while fidx[0] < len(fillers) and fillers[fidx[0]][0] <= (p, i):
                        fillers[fidx[0]][1]()
                        fidx[0] += 1
                    while av_queue and av_queue[0][0] <= i:
                        av_queue.pop(0)[1]()
                return pump

            # pre-phase: KQ(0) first half, narrowest-first — row 0 of pair 0
            # needs only the first 128 columns, so scoring starts as soon as
            # the weight DMAs land.
            kq_span(0, 0, 0, 128)
            kq_span(0, 1, 0, 128)
            kq_span(0, 0, 128, 512)
            kq_span(0, 1, 128, 512)

            for p in range(8):
                if p == 7:
                    # Self-host AV(7), but only from row 5 on: the za PSUM ring
                    # slots are still owned by AV(6) (drained via the pump at
                    # rows <= 5); claiming them earlier would head-of-line
                    # block the PE queue on AV(6)'s own not-yet-issued matmuls.
                    st7 = {}

                    def scale7(ii, heads, scat):
                        rcp2 = stats.tile([P, 2], f32, tag="rcp2", name=f"r7_{ii}")
                        for k in range(2):
                            nc.vector.reciprocal(
                                rcp2[:, k:k + 1],
                                scat[:, k * 8 + ii:k * 8 + ii + 1])
                        for k, (a, off) in enumerate(heads):
                            nc.vector.tensor_scalar_mul(
                                V[:, ii, a * H:(a + 1) * H],
                                V[:, ii, a * H:(a + 1) * H],
                                rcp2[:, k:k + 1],
                            )

                    def row7_done(i, heads, Ets, scat):
                        if i < 5:
                            return
                        first = range(6) if i == 5 else [i]
                        for ii in first:
                            scale7(ii, heads, scat)
                            av_step(7, heads, Ets, st7, ii)

                    heads, Ets, scat = attn_rows(p, pump_factory(p), row7_done)
                    nc.vector.tensor_copy(out=zT[:, 7, 0:512], in_=st7[0][:])
                    nc.vector.tensor_copy(out=zT[:, 7, 512:1024], in_=st7[1][:])
                else:
                    heads, Ets, scat = attn_rows(p, pump_factory(p))
                    av_queue = av_closures(p, heads, Ets, scat)
            while fidx[0] < len(fillers):
                fillers[fidx[0]][1]()
                fidx[0] += 1

            # ---- output projection, second half + combine ----
            # One 2-bank PSUM tile per m-tile (from the now-idle score pool),
            # a single wide tensor_tensor add against the parked first half,
            # and a single output DMA — the tail is PE-paced, not DVE/DMA-
            # issue-paced.
            for m in range(CT):
                ps = pss.tile([P, 1024], f32, tag="srow", name=f"ob{m}")
                for n_ in range(NC):
                    for p2 in range(4, ET):
                        nc.tensor.matmul(
                            ps[:, n_ * 512:(n_ + 1) * 512],
                            zT[:, p2, m * P:(m + 1) * P],
                            wo[:, p2, n_ * 512:(n_ + 1) * 512],
                            start=(p2 == 4), stop=(p2 == ET - 1),
                        )
                ot = outp.tile([P, 1024], f16, tag="ot")
                nc.vector.tensor_tensor(
                    ot[:], ps[:], oacc[:, m, :], mybir.AluOpType.add,
                )
                nc.sync.dma_start(out_d[m * P:(m + 1) * P, :], ot[:])

    # HW allows only one sync-wait per instruction (matmuls especially);
    # split excess waits into InstEventSemaphore like the bacc layer does.
    import bass_rust
    bass_rust.generate_event_semaphores(nc)
    return nc


def _host_prep(x, W_K, W_Q, W_V, W_O):
    """Pack per-core input dicts (host-side layout prep, fp16 casts)."""
    wk = W_K.transpose(2, 0, 1).reshape(E, A * H)
    wq = (W_Q / np.sqrt(H)).transpose(2, 0, 1).reshape(E, A * H)
    wv = W_V.transpose(2, 0, 1).reshape(E, A * H)
    wkqv = np.concatenate([wk, wq, wv], axis=1).astype(np.float16)
    wo = np.ascontiguousarray(W_O.T).astype(np.float16)

    r = np.arange(P)[:, None]
    d = np.arange(P)[None, :]
    msk = np.where(d <= r, 0.0, NEG).astype(np.float16)   # causal 128-block
    ident = np.eye(P, dtype=np.float16)

    in_maps = []
    for b in range(B):
        in_maps.append({
            "xt": np.ascontiguousarray(x[b].T).astype(np.float16),
            "wkqv": wkqv,
            "wo": wo,
            "msk": msk,
            "ident": ident,
        })
    return in_maps


def _run(x, W_K, W_Q, W_V, W_O, **spmd_kwargs):
    from concourse.bass_utils import run_bass_kernel_spmd

    if "nc" not in _cache:
        _cache["nc"] = _build_nc()
    in_maps = _host_prep(
        np.asarray(x, dtype=np.float32), np.asarray(W_K, dtype=np.float32),
        np.asarray(W_Q, dtype=np.float32), np.asarray(W_V, dtype=np.float32),
        np.asarray(W_O, dtype=np.float32),
    )
    res = run_bass_kernel_spmd(_cache["nc"], in_maps, core_ids=list(range(B)),
                               **spmd_kwargs)
    out = np.stack([r["out"] for r in res.results], axis=0).astype(np.float32)
    return out, res


def kernel(x, W_K, W_Q, W_V, W_O):
    out, _ = _run(x, W_K, W_Q, W_V, W_O)
    return out


# revision 27
# speedup vs baseline: 1.1080x; 1.1080x over previous
"""Trainium2 Bass kernel for 16-head causal attention (transposed-softmax variant).

Problem shapes: x [8, 1024, 1024]; W_K/W_Q/W_V [16, 64, 1024]; W_O [1024, 1024].
Sharding: pure data-parallel over batch (8 batch elements -> 8 cores), weights
replicated, no collectives.

Per-core pipeline (one batch element, seq=1024, d_embed=1024, 16 heads x 64):
  1. QKV projections as K_T/Q_T [heads*64, seq] and V [seq, heads*64], fp16
     operands, fp32 PSUM accumulation. W_Q is pre-scaled by 1/sqrt(d_head).
  2. Per head pair: scores S[c, C] for causal-allowed chunks, two heads
     concurrent in disjoint 64-row PE groups; triangular diag mask accumulated
     via identity matmul.  Rows i<4 put both heads side by side in one 2-bank
     PSUM tile; rows i>=4 use one 2-bank tile per head.
  3. Softmax over C without max-subtraction: exp per (row, head) on ScalarE
     with accum_out row sums collected into a per-pair [128,16] tile; one
     reciprocal + two broadcast tensor_tensor muls fold the normalization into
     V rows (V' = V/rowsum), applied lazily at the start of the next pair.
  4. zT[f, C] += V'^T E ascending in i (start flags at i=0 / i=4 per 512-chunk,
     stop at i=7), lagged a half-pair (self-hosted per-row for the last pair).
  5. Output projection split along the contraction: out_A (zT pairs 0-3) runs
     as dense filler inside pairs 5-7 and parks in SBUF fp16; out_B (pairs 4-7)
     runs in the tail and is combined via tensor_tensor add.

Scheduling: the scalar engine (exp chain) paces each pair at ~20us while the
PE's own attention work is ~9us, so dense projection groups are drip-fed as
fillers into every pair with just-in-time deadlines.  This keeps PE activity
density high everywhere so the HAM clock gate stays at K=8/8 (a previous
version ran the whole second half at 1.2 GHz because fillers were exhausted
after pair 3).
"""

import numpy as np

S, E, A, H, B = 1024, 1024, 16, 64, 8
P = 128          # partitions
NEG = -30000.0   # additive mask value (fp16-safe; exp -> 0 in fp32)

_cache = {}


def _off(i):
    """Compact E-buffer offset of row-tile i (valid width of row i is (i+1)*P)."""
    return P * i * (i + 1) // 2


EW = _off(8)     # 4608 columns total


def _build_nc():
    import concourse.bass as bass
    import concourse.mybir as mybir
    from concourse.tile import TileContext

    f16 = mybir.dt.float16
    f32 = mybir.dt.float32
    Exp = mybir.ActivationFunctionType.Exp

    nc = bass.Bass()
    # inputs are pre-tiled on the host to [128, e-tile, cols] so one DMA can
    # load a column slice across all 8 e-tiles (the per-DMA issue cost is
    # ~650ns on the issuing engine; per-engine queue bandwidth ~150 GB/s)
    xt_d = nc.dram_tensor("xt", [P, E // P, S], f16, kind="ExternalInput")
    wkqv_d = nc.dram_tensor("wkqv", [P, E // P, 3 * A * H], f16,
                            kind="ExternalInput")
    wo_d = nc.dram_tensor("wo", [P, A * H // P, E], f16, kind="ExternalInput")
    msk_d = nc.dram_tensor("msk", [P, P], f16, kind="ExternalInput")
    id_d = nc.dram_tensor("ident", [P, P], f16, kind="ExternalInput")
    out_d = nc.dram_tensor("out", [S, E], f16, kind="ExternalOutput")

    ET = E // P       # 8 e-tiles
    CT = S // P       # 8 c-tiles
    NC = S // 512     # 2 512-chunks

    with TileContext(nc) as tc:
        with (
            tc.tile_pool(name="inp", bufs=1) as inp,
            tc.tile_pool(name="kqv", bufs=1) as kqv,
            tc.tile_pool(name="epool", bufs=4) as epool,
            tc.tile_pool(name="stats", bufs=4) as stats,
            tc.tile_pool(name="outp", bufs=3) as outp,
            tc.tile_pool(name="pss", bufs=2, space="PSUM") as pss,   # 4 banks
            tc.tile_pool(name="psq", bufs=2, space="PSUM") as psq,   # 2 banks
            tc.tile_pool(name="pza", bufs=2, space="PSUM") as pza,   # 2 banks
        ):
            # ---- SBUF destinations ----
            xT = inp.tile([P, ET, S], f16, tag="xT")
            wkqv = inp.tile([P, ET, 3 * A * H], f16, tag="wkqv")
            wo = inp.tile([P, ET, E], f16, tag="wo")
            msk = inp.tile([P, P], f16, tag="msk")
            ident = inp.tile([P, P], f16, tag="ident")
            warm = inp.tile([P, 512], f16, tag="warm")

            nc.any.memset(warm[:], 0.125)

            # ---- loads, ordered by first use.  The sync engine needs ~650ns
            # per DMA_DIRECT2D issue, so the critical prefix (x first half,
            # K/Q weights of pairs 0-3) is issued from three engines in
            # parallel; the rest dribbles out on sync in deadline order. ----
            AH = A * H
            # sync-engine stream, in deadline order
            nc.sync.dma_start(ident[:], id_d[:])
            nc.sync.dma_start(msk[:], msk_d[:])
            nc.sync.dma_start(xT[:, :, 0:128], xt_d[:, :, 0:128])      # c-tile 0
            nc.sync.dma_start(xT[:, :, 128:512], xt_d[:, :, 128:512])  # c-tiles 1-3
            nc.sync.dma_start(wkqv[:, :, 0:512], wkqv_d[:, :, 0:512])  # K, pairs 0-3
            nc.sync.dma_start(wkqv[:, :, 2 * AH:2 * AH + 512],         # V, f 0:512
                              wkqv_d[:, :, 2 * AH:2 * AH + 512])
            nc.sync.dma_start(wkqv[:, :, 512:AH], wkqv_d[:, :, 512:AH])  # K, 4-7
            nc.sync.dma_start(wkqv[:, :, 2 * AH + 512:3 * AH],         # V, f 512:
                              wkqv_d[:, :, 2 * AH + 512:3 * AH])
            # scalar-engine stream, in deadline order
            nc.scalar.dma_start(wkqv[:, :, AH:AH + 512],               # Q, pairs 0-3
                                wkqv_d[:, :, AH:AH + 512])
            nc.scalar.dma_start(xT[:, :, 512:S], xt_d[:, :, 512:S])    # x 2nd half
            nc.scalar.dma_start(wkqv[:, :, AH + 512:2 * AH],           # Q, pairs 4-7
                                wkqv_d[:, :, AH + 512:2 * AH])
            nc.scalar.dma_start(wo[:, :, :], wo_d[:, :, :])            # out proj

            K_T = kqv.tile([P, A // 2, S], f16, tag="K_T")   # pair-stacked [2h, c]
            Q_T = kqv.tile([P, A // 2, S], f16, tag="Q_T")
            V = kqv.tile([P, CT, A * H], f16, tag="V")       # [c, f]
            zT = kqv.tile([P, A // 2, S], f16, tag="zT")     # pair-stacked [f, C]
            oacc = kqv.tile([P, CT, E], f16, tag="oacc")     # out_A parking

            # ---- PE warm-up: ride out the HAM throttle during the DMA wait ----
            wps = psq.tile([P, 512], f32, tag="psq", name="warm")
            for w in range(28):
                nc.tensor.matmul(wps[:], ident[:], warm[:],
                                 start=(w == 0), stop=(w == 27),
                                 skip_group_check=True)
            wsb = stats.tile([P, 1], f32, tag="wsink", name="warmsink")
            nc.vector.reduce_max(wsb[:], wps[:, :P], axis=mybir.AxisListType.X)

            # ---- dense projection groups (the filler pool) ----
            def kq_span(p, mat, lo, hi):
                dst = K_T if mat == 0 else Q_T
                w = hi - lo
                ps = psq.tile([P, 512], f32, tag="psq", name=f"q{p}{mat}{lo}")
                for et in range(ET):
                    nc.tensor.matmul(
                        ps[:, 0:w],
                        wkqv[:, et, mat * A * H + p * P: mat * A * H + (p + 1) * P],
                        xT[:, et, lo:hi],
                        start=(et == 0), stop=(et == ET - 1),
                    )
                nc.vector.tensor_copy(out=dst[:, p, lo:hi], in_=ps[:, 0:w])

            def kq_group(p, mat, cc):
                kq_span(p, mat, cc * 512, (cc + 1) * 512)

            def v_group(fc, i):
                ps = psq.tile([P, 512], f32, tag="psq", name=f"v{fc}{i}")
                for et in range(ET):
                    nc.tensor.matmul(
                        ps[:],
                        xT[:, et, i * P:(i + 1) * P],
                        wkqv[:, et, 2 * A * H + fc * 512: 2 * A * H + (fc + 1) * 512],
                        start=(et == 0), stop=(et == ET - 1),
                    )
                nc.vector.tensor_copy(out=V[:, i, fc * 512:(fc + 1) * 512], in_=ps[:])

            def oa_group(m, n_):
                """out_A: first-half contraction (zT pairs 0-3), parked fp16."""
                ps = psq.tile([P, 512], f32, tag="psq", name=f"oa{m}{n_}")
                for p2 in range(4):
                    nc.tensor.matmul(
                        ps[:],
                        zT[:, p2, m * P:(m + 1) * P],
                        wo[:, p2, n_ * 512:(n_ + 1) * 512],
                        start=(p2 == 0), stop=(p2 == 3),
                    )
                nc.vector.tensor_copy(out=oacc[:, m, n_ * 512:(n_ + 1) * 512], in_=ps[:])

            # ---- attention ----
            def attn_rows(p, pump, row_done=None):
                """Scores+exp rows of pair p.  pump(i) interleaves filler work
                after row i's exp chain; row_done(i, ...) optionally self-hosts
                AV steps (pair 7)."""
                heads = [(2 * p, 0), (2 * p + 1, H)]
                Ets = [epool.tile([P, EW], f16, tag="E", name=f"E{k}_{p}")
                       for k in range(2)]
                scat = stats.tile([P, 16], f32, tag="ssum", name=f"sc{p}")
                for i in range(CT):
                    vw = (i + 1) * P          # causally-valid row width
                    if i < 4:   # both heads side by side in one 2-bank tile
                        row = pss.tile([P, 1024], f32, tag="srow", name=f"r_{p}_{i}")
                        views = [row[:, 0:vw], row[:, 512:512 + vw]]
                        dviews = [row[:, i * P:(i + 1) * P],
                                  row[:, 512 + i * P:512 + i * P + P]]
                        n_i = 1
                    else:
                        rows = [pss.tile([P, 1024], f32, tag="srow",
                                         name=f"r{k}_{p}_{i}")
                                for k in range(2)]
                        views = [rows[0][:, 0:vw], rows[1][:, 0:vw]]
                        dviews = [rows[0][:, i * P:(i + 1) * P],
                                  rows[1][:, i * P:(i + 1) * P]]
                        n_i = 2
                    for j in range(n_i):
                        diag = j == n_i - 1
                        ntrim = vw - j * 512 if diag else 512
                        for k, (a, off) in enumerate(heads):
                            nc.tensor.matmul(
                                views[k][:, j * 512:j * 512 + ntrim],
                                K_T[off:off + H, p, i * P:(i + 1) * P],
                                Q_T[off:off + H, p, j * 512:j * 512 + ntrim],
                                start=True, stop=not diag,
                                skip_group_check=True,
                            )
                        if diag:
                            # causal mask of the diagonal block via identity
                            # matmul — keeps the scores->exp chain on the PE
                            # (a DVE tensor_tensor here serializes every row
                            # behind queued casts: +25us measured)
                            for k in range(2):
                                nc.tensor.matmul(
                                    dviews[k],
                                    ident[:], msk[:],
                                    start=False, stop=True,
                                    skip_group_check=True,
                                )
                    for k, (a, off) in enumerate(heads):
                        nc.scalar.activation(
                            Ets[k][:, _off(i):_off(i) + vw], views[k], Exp,
                            accum_out=scat[:, k * 8 + i:k * 8 + i + 1],
                        )
                    pump(i)
                    if row_done is not None:
                        row_done(i, heads, Ets, scat)
                return heads, Ets, scat

            def av_step(p, heads, Ets, state, i):
                """One ascending AV accumulation step (both 512-chunks)."""
                if i == 0:
                    state[0] = pza.tile([P, 512], f32, tag="za", name=f"za_{p}_0")
                if i == 4:
                    state[1] = pza.tile([P, 512], f32, tag="za", name=f"za_{p}_1")
                for j in range(NC):
                    if i < 4 * j:
                        continue
                    ntrim = min(512, (i + 1) * P - j * 512)
                    for k, (a, off) in enumerate(heads):
                        nc.tensor.matmul(
                            state[j][off:off + H, :ntrim],
                            V[:, i, a * H:(a + 1) * H],
                            Ets[k][:, _off(i) + j * 512:_off(i) + j * 512 + ntrim],
                            start=(i == 4 * j), stop=(i == CT - 1),
                            skip_group_check=True,
                        )

            def scale_all(p, heads, scat):
                """Fold 1/rowsum into V rows of pair p (all 8 row-tiles)."""
                rcp = stats.tile([P, 16], f32, tag="rcp", name=f"rc{p}")
                nc.vector.reciprocal(rcp[:], scat[:])
                for k, (a, off) in enumerate(heads):
                    vs = V[:, :, a * H:(a + 1) * H]
                    nc.vector.tensor_tensor(
                        vs, vs,
                        rcp[:, k * 8:(k + 1) * 8, None].to_broadcast([P, CT, H]),
                        mybir.AluOpType.mult,
                    )

            def av_closures(p, heads, Ets, scat):
                """Lazy normalize + AV of pair p, interleaved into pair p+1.
                Pair 0's chain is pushed to row 3+ so its V groups (gated on
                the V-weight DMA) get breathing room."""
                off = 3 if p == 0 else 0
                state = {}
                cs = [(off, lambda: scale_all(p, heads, scat))]
                for i in range(CT):
                    cs.append((min(7, off + (i + 1) // 2),
                               lambda i=i: av_step(p, heads, Ets, state, i)))
                cs.append((min(7, off + 4), lambda: nc.vector.tensor_copy(
                    out=zT[:, p, 0:512], in_=state[0][:])))
                cs.append((min(7, off + 5), lambda: nc.vector.tensor_copy(
                    out=zT[:, p, 512:1024], in_=state[1][:])))
                return cs

            # ---- filler schedule -------------------------------------------
            # Queue of (deadline (pair,row), closure); before row r of pair p
            # every filler with deadline <= (p, r) is forced out.
            fillers = []

            def add(dl, cl):
                fillers.append((dl, cl))

            # pair 0/1 KQ halves not covered by the (minimal) pre-phase
            add((0, 0), lambda: kq_group(1, 0, 0))
            add((0, 1), lambda: kq_group(1, 1, 0))
            add((0, 2), lambda: kq_group(0, 0, 1))
            add((0, 3), lambda: kq_group(0, 1, 1))
            add((0, 5), lambda: kq_group(1, 0, 1))
            add((0, 6), lambda: kq_group(1, 1, 1))
            for p in range(2, 8):
                add((p - 2, 4), lambda p=p: kq_group(p, 0, 0))
                add((p - 2, 6), lambda p=p: kq_group(p, 1, 0))
                add((p - 1, 1), lambda p=p: kq_group(p, 0, 1))
                add((p - 1, 2), lambda p=p: kq_group(p, 1, 1))
            # V(fc0, i) needed by scale_all(0) at pair-1 row 3;
            # V(fc1, i) by scale_all(4) at pair-5 row 0.
            v0_dl = [(0, 4), (0, 6), (0, 7), (1, 0), (1, 0), (1, 1), (1, 2), (1, 3)]
            for i in range(CT):
                add(v0_dl[i], lambda i=i: v_group(0, i))
            v1_dl = [(2, 1), (2, 3), (2, 5), (3, 1), (3, 3), (4, 1), (4, 3), (4, 5)]
            for i in range(CT):
                add(v1_dl[i], lambda i=i: v_group(1, i))
            # out_A(m, n) after AV(3) copies (pair-4 row 5); spread pairs 4-7.
            oa_dl = [(4, 6), (4, 7), (5, 1), (5, 3), (5, 5), (5, 7),
                     (6, 1), (6, 2), (6, 3), (6, 4), (6, 5), (6, 6),
                     (7, 1), (7, 3), (7, 5), (7, 6)]
            for m in range(CT):
                for n_ in range(NC):
                    add(oa_dl[2 * m + n_], lambda m=m, n_=n_: oa_group(m, n_))

            fillers.sort(key=lambda x: x[0])
            fidx = [0]
            av_queue = []   # (deadline_row, closure) of the previous pair's AV

            def pump_factory(p):
                def pump(i):
                    while fidx[0] < len(fillers) and fillers[fidx[0]][0] <= (p, i):
                        fillers[fidx[0]][1]()
                        fidx[0] += 1
                    while av_queue and av_queue[0][0] <= i:
                        av_queue.pop(0)[1]()
                return pump

            # pre-phase: KQ(0) first half, narrowest-first — row 0 of pair 0
            # needs only the first 128 columns, so scoring starts as soon as
            # the weight DMAs land.
            kq_span(0, 0, 0, 128)
            kq_span(0, 1, 0, 128)
            kq_span(0, 0, 128, 512)
            kq_span(0, 1, 128, 512)

            for p in range(8):
                if p == 7:
                    # Self-host AV(7), but only from row 5 on: the za PSUM ring
                    # slots are still owned by AV(6) (drained via the pump at
                    # rows <= 5); claiming them earlier would head-of-line
                    # block the PE queue on AV(6)'s own not-yet-issued matmuls.
                    st7 = {}

                    def scale7(ii, heads, scat):
                        rcp2 = stats.tile([P, 2], f32, tag="rcp2", name=f"r7_{ii}")
                        for k in range(2):
                            nc.vector.reciprocal(
                                rcp2[:, k:k + 1],
                                scat[:, k * 8 + ii:k * 8 + ii + 1])
                        for k, (a, off) in enumerate(heads):
                            nc.vector.tensor_scalar_mul(
                                V[:, ii, a * H:(a + 1) * H],
                                V[:, ii, a * H:(a + 1) * H],
                                rcp2[:, k:k + 1],
                            )

                    def row7_done(i, heads, Ets, scat):
                        if i < 5:
                            return
                        first = range(6) if i == 5 else [i]
                        for ii in first:
                            scale7(ii, heads, scat)
                            av_step(7, heads, Ets, st7, ii)

                    heads, Ets, scat = attn_rows(p, pump_factory(p), row7_done)
                    nc.vector.tensor_copy(out=zT[:, 7, 0:512], in_=st7[0][:])
                    nc.vector.tensor_copy(out=zT[:, 7, 512:1024], in_=st7[1][:])
                else:
                    heads, Ets, scat = attn_rows(p, pump_factory(p))
                    av_queue = av_closures(p, heads, Ets, scat)
            while fidx[0] < len(fillers):
                fillers[fidx[0]][1]()
                fidx[0] += 1

            # ---- output projection, second half + combine ----
            # One 2-bank PSUM tile per m-tile (from the now-idle score pool),
            # a single wide tensor_tensor add against the parked first half,
            # and a single output DMA — the tail is PE-paced, not DVE/DMA-
            # issue-paced.
            for m in range(CT):
                ps = pss.tile([P, 1024], f32, tag="srow", name=f"ob{m}")
                for n_ in range(NC):
                    for p2 in range(4, ET):
                        nc.tensor.matmul(
                            ps[:, n_ * 512:(n_ + 1) * 512],
                            zT[:, p2, m * P:(m + 1) * P],
                            wo[:, p2, n_ * 512:(n_ + 1) * 512],
                            start=(p2 == 4), stop=(p2 == ET - 1),
                        )
                ot = outp.tile([P, 1024], f16, tag="ot")
                nc.vector.tensor_tensor(
                    ot[:], ps[:], oacc[:, m, :], mybir.AluOpType.add,
                )
                nc.sync.dma_start(out_d[m * P:(m + 1) * P, :], ot[:])

    # HW allows only one sync-wait per instruction (matmuls especially);
    # split excess waits into InstEventSemaphore like the bacc layer does.
    import bass_rust
    bass_rust.generate_event_semaphores(nc)
    return nc


def _host_prep(x, W_K, W_Q, W_V, W_O):
    """Pack per-core input dicts (host-side layout prep, fp16 casts)."""
    wk = W_K.transpose(2, 0, 1).reshape(E, A * H)
    wq = (W_Q / np.sqrt(H)).transpose(2, 0, 1).reshape(E, A * H)
    wv = W_V.transpose(2, 0, 1).reshape(E, A * H)
    wkqv = np.concatenate([wk, wq, wv], axis=1).astype(np.float16)
    wo = np.ascontiguousarray(W_O.T).astype(np.float16)

    r = np.arange(P)[:, None]
    d = np.arange(P)[None, :]
    msk = np.where(d <= r, 0.0, NEG).astype(np.float16)   # causal 128-block
    ident = np.eye(P, dtype=np.float16)

    def tile_rows(m):
        # [R, C] -> [128, R//128, C]: row r = t*128 + p lands at [p, t, :]
        return np.ascontiguousarray(
            m.reshape(-1, P, m.shape[1]).transpose(1, 0, 2))

    wkqv_t = tile_rows(wkqv)
    wo_t = tile_rows(wo)
    in_maps = []
    for b in range(B):
        in_maps.append({
            "xt": tile_rows(np.ascontiguousarray(x[b].T).astype(np.float16)),
            "wkqv": wkqv_t,
            "wo": wo_t,
            "msk": msk,
            "ident": ident,
        })
    return in_maps


def _run(x, W_K, W_Q, W_V, W_O, **spmd_kwargs):
    from concourse.bass_utils import run_bass_kernel_spmd

    if "nc" not in _cache:
        _cache["nc"] = _build_nc()
    in_maps = _host_prep(
        np.asarray(x, dtype=np.float32), np.asarray(W_K, dtype=np.float32),
        np.asarray(W_Q, dtype=np.float32), np.asarray(W_V, dtype=np.float32),
        np.asarray(W_O, dtype=np.float32),
    )
    res = run_bass_kernel_spmd(_cache["nc"], in_maps, core_ids=list(range(B)),
                               **spmd_kwargs)
    out = np.stack([r["out"] for r in res.results], axis=0).astype(np.float32)
    return out, res


def kernel(x, W_K, W_Q, W_V, W_O):
    out, _ = _run(x, W_K, W_Q, W_V, W_O)
    return out


# revision 28
# speedup vs baseline: 1.1652x; 1.0516x over previous
"""Trainium2 Bass kernel for 16-head causal attention (transposed-softmax variant).

Problem shapes: x [8, 1024, 1024]; W_K/W_Q/W_V [16, 64, 1024]; W_O [1024, 1024].
Sharding: pure data-parallel over batch (8 batch elements -> 8 cores), weights
replicated, no collectives.

Per-core pipeline (one batch element, seq=1024, d_embed=1024, 16 heads x 64):
  1. QKV projections as K_T/Q_T [heads*64, seq] and V [seq, heads*64], fp16
     operands, fp32 PSUM accumulation. W_Q is pre-scaled by 1/sqrt(d_head).
  2. Per head pair: scores S[c, C] for causal-allowed chunks, two heads
     concurrent in disjoint 64-row PE groups; triangular diag mask accumulated
     via identity matmul.  Rows i<4 put both heads side by side in one 2-bank
     PSUM tile; rows i>=4 use one 2-bank tile per head.
  3. Softmax over C without max-subtraction: exp per (row, head) on ScalarE
     with accum_out row sums collected into a per-pair [128,16] tile; one
     reciprocal + two broadcast tensor_tensor muls fold the normalization into
     V rows (V' = V/rowsum), applied lazily at the start of the next pair.
  4. zT[f, C] += V'^T E ascending in i (start flags at i=0 / i=4 per 512-chunk,
     stop at i=7), lagged a half-pair (self-hosted per-row for the last pair).
  5. Output projection split along the contraction: out_A (zT pairs 0-3) runs
     as dense filler inside pairs 5-7 and parks in SBUF fp16; out_B (pairs 4-7)
     runs in the tail and is combined via tensor_tensor add.

Scheduling: the scalar engine (exp chain) paces each pair at ~20us while the
PE's own attention work is ~9us, so dense projection groups are drip-fed as
fillers into every pair with just-in-time deadlines.  This keeps PE activity
density high everywhere so the HAM clock gate stays at K=8/8 (a previous
version ran the whole second half at 1.2 GHz because fillers were exhausted
after pair 3).
"""

import numpy as np

S, E, A, H, B = 1024, 1024, 16, 64, 8
P = 128          # partitions
NEG = -30000.0   # additive mask value (fp16-safe; exp -> 0 in fp32)

_cache = {}


def _off(i):
    """Compact E-buffer offset of row-tile i (valid width of row i is (i+1)*P)."""
    return P * i * (i + 1) // 2


EW = _off(8)     # 4608 columns total


def _build_nc():
    import concourse.bass as bass
    import concourse.mybir as mybir
    from concourse.tile import TileContext

    f16 = mybir.dt.float16
    f32 = mybir.dt.float32
    Exp = mybir.ActivationFunctionType.Exp

    nc = bass.Bass()
    # inputs are pre-tiled on the host to [128, e-tile, cols] so one DMA can
    # load a column slice across all 8 e-tiles (the per-DMA issue cost is
    # ~650ns on the issuing engine; per-engine queue bandwidth ~150 GB/s)
    xt_d = nc.dram_tensor("xt", [P, E // P, S], f16, kind="ExternalInput")
    wkqv_d = nc.dram_tensor("wkqv", [P, E // P, 3 * A * H], f16,
                            kind="ExternalInput")
    wo_d = nc.dram_tensor("wo", [P, A * H // P, E], f16, kind="ExternalInput")
    msk_d = nc.dram_tensor("msk", [P, P], f16, kind="ExternalInput")
    id_d = nc.dram_tensor("ident", [P, P], f16, kind="ExternalInput")
    out_d = nc.dram_tensor("out", [S, E], f16, kind="ExternalOutput")

    ET = E // P       # 8 e-tiles
    CT = S // P       # 8 c-tiles
    NC = S // 512     # 2 512-chunks

    with TileContext(nc) as tc:
        with (
            tc.tile_pool(name="inp", bufs=1) as inp,
            tc.tile_pool(name="kqv", bufs=1) as kqv,
            tc.tile_pool(name="epool", bufs=4) as epool,
            tc.tile_pool(name="stats", bufs=4) as stats,
            tc.tile_pool(name="outp", bufs=3) as outp,
            tc.tile_pool(name="pss", bufs=2, space="PSUM") as pss,   # 4 banks
            tc.tile_pool(name="psq", bufs=2, space="PSUM") as psq,   # 2 banks
            tc.tile_pool(name="pza", bufs=2, space="PSUM") as pza,   # 2 banks
        ):
            # ---- SBUF destinations ----
            xT = inp.tile([P, ET, S], f16, tag="xT")
            wkqv = inp.tile([P, ET, 3 * A * H], f16, tag="wkqv")
            wo = inp.tile([P, ET, E], f16, tag="wo")
            msk = inp.tile([P, P], f16, tag="msk")
            ident = inp.tile([P, P], f16, tag="ident")
            warm = inp.tile([P, 512], f16, tag="warm")

            nc.any.memset(warm[:], 0.125)

            # ---- loads, ordered by first use.  The sync engine needs ~650ns
            # per DMA_DIRECT2D issue, so the critical prefix (x first half,
            # K/Q weights of pairs 0-3) is issued from three engines in
            # parallel; the rest dribbles out on sync in deadline order. ----
            AH = A * H
            # Per-e-tile DMAs: one monolithic [128, 8, cols] DMA runs on a
            # single DMA engine (~90-130 GB/s) and blocks its queue; eight
            # [128, cols] DMAs fan out across DMA engines (~160 GB/s/queue).
            # Two issuing engines (sync + scalar) give two parallel queues.
            nc.sync.dma_start(ident[:], id_d[:])
            nc.sync.dma_start(msk[:], msk_d[:])
            for t in range(ET):                       # x first 512 c's
                nc.sync.dma_start(xT[:, t, 0:512], xt_d[:, t, 0:512])
            for t in range(ET):                       # K weights, pairs 0-3
                nc.scalar.dma_start(wkqv[:, t, 0:512], wkqv_d[:, t, 0:512])
            for t in range(ET):                       # Q weights, pairs 0-3
                eng = nc.scalar if t < 4 else nc.sync
                eng.dma_start(wkqv[:, t, AH:AH + 512],
                              wkqv_d[:, t, AH:AH + 512])
            for t in range(ET):                       # x second half
                nc.sync.dma_start(xT[:, t, 512:S], xt_d[:, t, 512:S])
            for t in range(ET):                       # V weights, f 0:512
                nc.sync.dma_start(wkqv[:, t, 2 * AH:2 * AH + 512],
                                  wkqv_d[:, t, 2 * AH:2 * AH + 512])
            for t in range(ET):                       # K weights, pairs 4-7
                nc.sync.dma_start(wkqv[:, t, 512:AH], wkqv_d[:, t, 512:AH])
            for t in range(ET):                       # Q weights, pairs 4-7
                nc.sync.dma_start(wkqv[:, t, AH + 512:2 * AH],
                                  wkqv_d[:, t, AH + 512:2 * AH])
            for t in range(ET):                       # V weights, f 512:1024
                nc.sync.dma_start(wkqv[:, t, 2 * AH + 512:3 * AH],
                                  wkqv_d[:, t, 2 * AH + 512:3 * AH])
            for t in range(ET):                       # output projection weights
                nc.sync.dma_start(wo[:, t, :], wo_d[:, t, :])

            K_T = kqv.tile([P, A // 2, S], f16, tag="K_T")   # pair-stacked [2h, c]
            Q_T = kqv.tile([P, A // 2, S], f16, tag="Q_T")
            V = kqv.tile([P, CT, A * H], f16, tag="V")       # [c, f]
            zT = kqv.tile([P, A // 2, S], f16, tag="zT")     # pair-stacked [f, C]
            oacc = kqv.tile([P, CT, E], f16, tag="oacc")     # out_A parking

            # ---- PE warm-up: ride out the HAM throttle during the DMA wait ----
            wps = psq.tile([P, 512], f32, tag="psq", name="warm")
            for w in range(28):
                nc.tensor.matmul(wps[:], ident[:], warm[:],
                                 start=(w == 0), stop=(w == 27),
                                 skip_group_check=True)
            wsb = stats.tile([P, 1], f32, tag="wsink", name="warmsink")
            nc.vector.reduce_max(wsb[:], wps[:, :P], axis=mybir.AxisListType.X)

            # ---- dense projection groups (the filler pool) ----
            def kq_span(p, mat, lo, hi):
                dst = K_T if mat == 0 else Q_T
                w = hi - lo
                ps = psq.tile([P, 512], f32, tag="psq", name=f"q{p}{mat}{lo}")
                for et in range(ET):
                    nc.tensor.matmul(
                        ps[:, 0:w],
                        wkqv[:, et, mat * A * H + p * P: mat * A * H + (p + 1) * P],
                        xT[:, et, lo:hi],
                        start=(et == 0), stop=(et == ET - 1),
                    )
                nc.vector.tensor_copy(out=dst[:, p, lo:hi], in_=ps[:, 0:w])

            def kq_group(p, mat, cc):
                kq_span(p, mat, cc * 512, (cc + 1) * 512)

            def v_group(fc, i):
                ps = psq.tile([P, 512], f32, tag="psq", name=f"v{fc}{i}")
                for et in range(ET):
                    nc.tensor.matmul(
                        ps[:],
                        xT[:, et, i * P:(i + 1) * P],
                        wkqv[:, et, 2 * A * H + fc * 512: 2 * A * H + (fc + 1) * 512],
                        start=(et == 0), stop=(et == ET - 1),
                    )
                nc.vector.tensor_copy(out=V[:, i, fc * 512:(fc + 1) * 512], in_=ps[:])

            def oa_group(m, n_):
                """out_A: first-half contraction (zT pairs 0-3), parked fp16."""
                ps = psq.tile([P, 512], f32, tag="psq", name=f"oa{m}{n_}")
                for p2 in range(4):
                    nc.tensor.matmul(
                        ps[:],
                        zT[:, p2, m * P:(m + 1) * P],
                        wo[:, p2, n_ * 512:(n_ + 1) * 512],
                        start=(p2 == 0), stop=(p2 == 3),
                    )
                nc.vector.tensor_copy(out=oacc[:, m, n_ * 512:(n_ + 1) * 512], in_=ps[:])

            # ---- attention ----
            def attn_rows(p, pump, row_done=None):
                """Scores+exp rows of pair p.  pump(i) interleaves filler work
                after row i's exp chain; row_done(i, ...) optionally self-hosts
                AV steps (pair 7)."""
                heads = [(2 * p, 0), (2 * p + 1, H)]
                Ets = [epool.tile([P, EW], f16, tag="E", name=f"E{k}_{p}")
                       for k in range(2)]
                scat = stats.tile([P, 16], f32, tag="ssum", name=f"sc{p}")
                for i in range(CT):
                    vw = (i + 1) * P          # causally-valid row width
                    if i < 4:   # both heads side by side in one 2-bank tile
                        row = pss.tile([P, 1024], f32, tag="srow", name=f"r_{p}_{i}")
                        views = [row[:, 0:vw], row[:, 512:512 + vw]]
                        dviews = [row[:, i * P:(i + 1) * P],
                                  row[:, 512 + i * P:512 + i * P + P]]
                        n_i = 1
                    else:
                        rows = [pss.tile([P, 1024], f32, tag="srow",
                                         name=f"r{k}_{p}_{i}")
                                for k in range(2)]
                        views = [rows[0][:, 0:vw], rows[1][:, 0:vw]]
                        dviews = [rows[0][:, i * P:(i + 1) * P],
                                  rows[1][:, i * P:(i + 1) * P]]
                        n_i = 2
                    for j in range(n_i):
                        diag = j == n_i - 1
                        ntrim = vw - j * 512 if diag else 512
                        for k, (a, off) in enumerate(heads):
                            nc.tensor.matmul(
                                views[k][:, j * 512:j * 512 + ntrim],
                                K_T[off:off + H, p, i * P:(i + 1) * P],
                                Q_T[off:off + H, p, j * 512:j * 512 + ntrim],
                                start=True, stop=not diag,
                                skip_group_check=True,
                            )
                        if diag:
                            # causal mask of the diagonal block via identity
                            # matmul — keeps the scores->exp chain on the PE
                            # (a DVE tensor_tensor here serializes every row
                            # behind queued casts: +25us measured)
                            for k in range(2):
                                nc.tensor.matmul(
                                    dviews[k],
                                    ident[:], msk[:],
                                    start=False, stop=True,
                                    skip_group_check=True,
                                )
                    for k, (a, off) in enumerate(heads):
                        nc.scalar.activation(
                            Ets[k][:, _off(i):_off(i) + vw], views[k], Exp,
                            accum_out=scat[:, k * 8 + i:k * 8 + i + 1],
                        )
                    pump(i)
                    if row_done is not None:
                        row_done(i, heads, Ets, scat)
                return heads, Ets, scat

            def av_step(p, heads, Ets, state, i):
                """One ascending AV accumulation step (both 512-chunks)."""
                if i == 0:
                    state[0] = pza.tile([P, 512], f32, tag="za", name=f"za_{p}_0")
                if i == 4:
                    state[1] = pza.tile([P, 512], f32, tag="za", name=f"za_{p}_1")
                for j in range(NC):
                    if i < 4 * j:
                        continue
                    ntrim = min(512, (i + 1) * P - j * 512)
                    for k, (a, off) in enumerate(heads):
                        nc.tensor.matmul(
                            state[j][off:off + H, :ntrim],
                            V[:, i, a * H:(a + 1) * H],
                            Ets[k][:, _off(i) + j * 512:_off(i) + j * 512 + ntrim],
                            start=(i == 4 * j), stop=(i == CT - 1),
                            skip_group_check=True,
                        )

            def scale_all(p, heads, scat):
                """Fold 1/rowsum into V rows of pair p (all 8 row-tiles)."""
                rcp = stats.tile([P, 16], f32, tag="rcp", name=f"rc{p}")
                nc.vector.reciprocal(rcp[:], scat[:])
                for k, (a, off) in enumerate(heads):
                    vs = V[:, :, a * H:(a + 1) * H]
                    nc.vector.tensor_tensor(
                        vs, vs,
                        rcp[:, k * 8:(k + 1) * 8, None].to_broadcast([P, CT, H]),
                        mybir.AluOpType.mult,
                    )

            def av_closures(p, heads, Ets, scat):
                """Lazy normalize + AV of pair p, interleaved into pair p+1.
                Pair 0's chain is pushed to row 3+ so its V groups (gated on
                the V-weight DMA) get breathing room."""
                off = 3 if p == 0 else 0
                state = {}
                cs = [(off, lambda: scale_all(p, heads, scat))]
                for i in range(CT):
                    cs.append((min(7, off + (i + 1) // 2),
                               lambda i=i: av_step(p, heads, Ets, state, i)))
                cs.append((min(7, off + 4), lambda: nc.vector.tensor_copy(
                    out=zT[:, p, 0:512], in_=state[0][:])))
                cs.append((min(7, off + 5), lambda: nc.vector.tensor_copy(
                    out=zT[:, p, 512:1024], in_=state[1][:])))
                return cs

            # ---- filler schedule -------------------------------------------
            # Queue of (deadline (pair,row), closure); before row r of pair p
            # every filler with deadline <= (p, r) is forced out.
            fillers = []

            def add(dl, cl):
                fillers.append((dl, cl))

            # pair 0/1 KQ halves not covered by the (minimal) pre-phase
            add((0, 0), lambda: kq_group(1, 0, 0))
            add((0, 1), lambda: kq_group(1, 1, 0))
            add((0, 2), lambda: kq_group(0, 0, 1))
            add((0, 3), lambda: kq_group(0, 1, 1))
            add((0, 5), lambda: kq_group(1, 0, 1))
            add((0, 6), lambda: kq_group(1, 1, 1))
            for p in range(2, 8):
                add((p - 2, 4), lambda p=p: kq_group(p, 0, 0))
                add((p - 2, 6), lambda p=p: kq_group(p, 1, 0))
                add((p - 1, 1), lambda p=p: kq_group(p, 0, 1))
                add((p - 1, 2), lambda p=p: kq_group(p, 1, 1))
            # V(fc0, i) needed by scale_all(0) at pair-1 row 3;
            # V(fc1, i) by scale_all(4) at pair-5 row 0.
            v0_dl = [(0, 4), (0, 6), (0, 7), (1, 0), (1, 0), (1, 1), (1, 2), (1, 3)]
            for i in range(CT):
                add(v0_dl[i], lambda i=i: v_group(0, i))
            v1_dl = [(2, 1), (2, 3), (2, 5), (3, 1), (3, 3), (4, 1), (4, 3), (4, 5)]
            for i in range(CT):
                add(v1_dl[i], lambda i=i: v_group(1, i))
            # out_A(m, n) after AV(3) copies (pair-4 row 5); spread pairs 4-7.
            oa_dl = [(4, 6), (4, 7), (5, 1), (5, 3), (5, 5), (5, 7),
                     (6, 1), (6, 2), (6, 3), (6, 4), (6, 5), (6, 6),
                     (7, 1), (7, 3), (7, 5), (7, 6)]
            for m in range(CT):
                for n_ in range(NC):
                    add(oa_dl[2 * m + n_], lambda m=m, n_=n_: oa_group(m, n_))

            fillers.sort(key=lambda x: x[0])
            fidx = [0]
            av_queue = []   # (deadline_row, closure) of the previous pair's AV

            def pump_factory(p):
                def pump(i):
                    while fidx[0] < len(fillers) and fillers[fidx[0]][0] <= (p, i):
                        fillers[fidx[0]][1]()
                        fidx[0] += 1
                    while av_queue and av_queue[0][0] <= i:
                        av_queue.pop(0)[1]()
                return pump

            # pre-phase: KQ(0) first half, narrowest-first — row 0 of pair 0
            # needs only the first 128 columns, so scoring starts as soon as
            # the weight DMAs land.
            kq_span(0, 0, 0, 128)
            kq_span(0, 1, 0, 128)
            kq_span(0, 0, 128, 512)
            kq_span(0, 1, 128, 512)

            for p in range(8):
                if p == 7:
                    # Self-host AV(7), but only from row 5 on: the za PSUM ring
                    # slots are still owned by AV(6) (drained via the pump at
                    # rows <= 5); claiming them earlier would head-of-line
                    # block the PE queue on AV(6)'s own not-yet-issued matmuls.
                    st7 = {}

                    def scale7(ii, heads, scat):
                        rcp2 = stats.tile([P, 2], f32, tag="rcp2", name=f"r7_{ii}")
                        for k in range(2):
                            nc.vector.reciprocal(
                                rcp2[:, k:k + 1],
                                scat[:, k * 8 + ii:k * 8 + ii + 1])
                        for k, (a, off) in enumerate(heads):
                            nc.vector.tensor_scalar_mul(
                                V[:, ii, a * H:(a + 1) * H],
                                V[:, ii, a * H:(a + 1) * H],
                                rcp2[:, k:k + 1],
                            )

                    def row7_done(i, heads, Ets, scat):
                        if i < 5:
                            return
                        first = range(6) if i == 5 else [i]
                        for ii in first:
                            scale7(ii, heads, scat)
                            av_step(7, heads, Ets, st7, ii)

                    heads, Ets, scat = attn_rows(p, pump_factory(p), row7_done)
                    nc.vector.tensor_copy(out=zT[:, 7, 0:512], in_=st7[0][:])
                    nc.vector.tensor_copy(out=zT[:, 7, 512:1024], in_=st7[1][:])
                else:
                    heads, Ets, scat = attn_rows(p, pump_factory(p))
                    av_queue = av_closures(p, heads, Ets, scat)
            while fidx[0] < len(fillers):
                fillers[fidx[0]][1]()
                fidx[0] += 1

            # ---- output projection, second half + combine ----
            # One 2-bank PSUM tile per m-tile (from the now-idle score pool),
            # a single wide tensor_tensor add against the parked first half,
            # and a single output DMA — the tail is PE-paced, not DVE/DMA-
            # issue-paced.
            for m in range(CT):
                ps = pss.tile([P, 1024], f32, tag="srow", name=f"ob{m}")
                for n_ in range(NC):
                    for p2 in range(4, ET):
                        nc.tensor.matmul(
                            ps[:, n_ * 512:(n_ + 1) * 512],
                            zT[:, p2, m * P:(m + 1) * P],
                            wo[:, p2, n_ * 512:(n_ + 1) * 512],
                            start=(p2 == 4), stop=(p2 == ET - 1),
                        )
                ot = outp.tile([P, 1024], f16, tag="ot")
                nc.vector.tensor_tensor(
                    ot[:], ps[:], oacc[:, m, :], mybir.AluOpType.add,
                )
                nc.sync.dma_start(out_d[m * P:(m + 1) * P, :], ot[:])

    # HW allows only one sync-wait per instruction (matmuls especially);
    # split excess waits into InstEventSemaphore like the bacc layer does.
    import bass_rust
    bass_rust.generate_event_semaphores(nc)
    return nc


def _host_prep(x, W_K, W_Q, W_V, W_O):
    """Pack per-core input dicts (host-side layout prep, fp16 casts)."""
    wk = W_K.transpose(2, 0, 1).reshape(E, A * H)
    wq = (W_Q / np.sqrt(H)).transpose(2, 0, 1).reshape(E, A * H)
    wv = W_V.transpose(2, 0, 1).reshape(E, A * H)
    wkqv = np.concatenate([wk, wq, wv], axis=1).astype(np.float16)
    wo = np.ascontiguousarray(W_O.T).astype(np.float16)

    r = np.arange(P)[:, None]
    d = np.arange(P)[None, :]
    msk = np.where(d <= r, 0.0, NEG).astype(np.float16)   # causal 128-block
    ident = np.eye(P, dtype=np.float16)

    def tile_rows(m):
        # [R, C] -> [128, R//128, C]: row r = t*128 + p lands at [p, t, :]
        return np.ascontiguousarray(
            m.reshape(-1, P, m.shape[1]).transpose(1, 0, 2))

    wkqv_t = tile_rows(wkqv)
    wo_t = tile_rows(wo)
    in_maps = []
    for b in range(B):
        in_maps.append({
            "xt": tile_rows(np.ascontiguousarray(x[b].T).astype(np.float16)),
            "wkqv": wkqv_t,
            "wo": wo_t,
            "msk": msk,
            "ident": ident,
        })
    return in_maps


def _run(x, W_K, W_Q, W_V, W_O, **spmd_kwargs):
    from concourse.bass_utils import run_bass_kernel_spmd

    if "nc" not in _cache:
        _cache["nc"] = _build_nc()
    in_maps = _host_prep(
        np.asarray(x, dtype=np.float32), np.asarray(W_K, dtype=np.float32),
        np.asarray(W_Q, dtype=np.float32), np.asarray(W_V, dtype=np.float32),
        np.asarray(W_O, dtype=np.float32),
    )
    res = run_bass_kernel_spmd(_cache["nc"], in_maps, core_ids=list(range(B)),
                               **spmd_kwargs)
    out = np.stack([r["out"] for r in res.results], axis=0).astype(np.float32)
    return out, res


def kernel(x, W_K, W_Q, W_V, W_O):
    out, _ = _run(x, W_K, W_Q, W_V, W_O)
    return out


# revision 33
# speedup vs baseline: 1.1735x; 1.0071x over previous
"""Trainium2 Bass kernel for 16-head causal attention (transposed-softmax variant).

Problem shapes: x [8, 1024, 1024]; W_K/W_Q/W_V [16, 64, 1024]; W_O [1024, 1024].
Sharding: pure data-parallel over batch (8 batch elements -> 8 cores), weights
replicated, no collectives.

Per-core pipeline (one batch element, seq=1024, d_embed=1024, 16 heads x 64):
  1. QKV projections as K_T/Q_T [heads*64, seq] and V [seq, heads*64], fp16
     operands, fp32 PSUM accumulation. W_Q is pre-scaled by 1/sqrt(d_head).
  2. Per head pair: scores S[c, C] for causal-allowed chunks, two heads
     concurrent in disjoint 64-row PE groups; triangular diag mask accumulated
     via identity matmul.  Rows i<4 put both heads side by side in one 2-bank
     PSUM tile; rows i>=4 use one 2-bank tile per head.
  3. Softmax over C without max-subtraction: exp per (row, head) on ScalarE
     with accum_out row sums collected into a per-pair [128,16] tile; one
     reciprocal + two broadcast tensor_tensor muls fold the normalization into
     V rows (V' = V/rowsum), applied lazily at the start of the next pair.
  4. zT[f, C] += V'^T E ascending in i (start flags at i=0 / i=4 per 512-chunk,
     stop at i=7), lagged a half-pair (self-hosted per-row for the last pair).
  5. Output projection split along the contraction: out_A (zT pairs 0-3) runs
     as dense filler inside pairs 5-7 and parks in SBUF fp16; out_B (pairs 4-7)
     runs in the tail and is combined via tensor_tensor add.

Scheduling: the scalar engine (exp chain) paces each pair at ~20us while the
PE's own attention work is ~9us, so dense projection groups are drip-fed as
fillers into every pair with just-in-time deadlines.  This keeps PE activity
density high everywhere so the HAM clock gate stays at K=8/8 (a previous
version ran the whole second half at 1.2 GHz because fillers were exhausted
after pair 3).
"""

import numpy as np

S, E, A, H, B = 1024, 1024, 16, 64, 8
P = 128          # partitions
NEG = -30000.0   # additive mask value (fp16-safe; exp -> 0 in fp32)

_cache = {}


def _off(i):
    """Compact E-buffer offset of row-tile i (valid width of row i is (i+1)*P)."""
    return P * i * (i + 1) // 2


EW = _off(8)     # 4608 columns total


def _build_nc():
    import concourse.bass as bass
    import concourse.mybir as mybir
    from concourse.tile import TileContext

    f16 = mybir.dt.float16
    f32 = mybir.dt.float32
    Exp = mybir.ActivationFunctionType.Exp

    nc = bass.Bass()
    xt_d = nc.dram_tensor("xt", [E, S], f16, kind="ExternalInput")        # x[b].T
    wkqv_d = nc.dram_tensor("wkqv", [E, 3 * A * H], f16, kind="ExternalInput")
    wo_d = nc.dram_tensor("wo", [A * H, E], f16, kind="ExternalInput")    # W_O.T
    msk_d = nc.dram_tensor("msk", [P, P], f16, kind="ExternalInput")
    id_d = nc.dram_tensor("ident", [P, P], f16, kind="ExternalInput")
    out_d = nc.dram_tensor("out", [S, E], f16, kind="ExternalOutput")

    ET = E // P       # 8 e-tiles
    CT = S // P       # 8 c-tiles
    NC = S // 512     # 2 512-chunks

    with TileContext(nc) as tc:
        with (
            tc.tile_pool(name="inp", bufs=1) as inp,
            tc.tile_pool(name="kqv", bufs=1) as kqv,
            tc.tile_pool(name="epool", bufs=4) as epool,
            tc.tile_pool(name="stats", bufs=4) as stats,
            tc.tile_pool(name="outp", bufs=3) as outp,
            tc.tile_pool(name="pss", bufs=2, space="PSUM") as pss,   # 4 banks
            tc.tile_pool(name="psq", bufs=2, space="PSUM") as psq,   # 2 banks
            tc.tile_pool(name="pza", bufs=2, space="PSUM") as pza,   # 2 banks
        ):
            # ---- SBUF destinations ----
            xT = inp.tile([P, ET, S], f16, tag="xT")
            wkqv = inp.tile([P, ET, 3 * A * H], f16, tag="wkqv")
            wo = inp.tile([P, ET, E], f16, tag="wo")
            msk = inp.tile([P, P], f16, tag="msk")
            ident = inp.tile([P, P], f16, tag="ident")
            warm = inp.tile([P, 512], f16, tag="warm")

            nc.any.memset(warm[:], 0.125)

            # ---- loads, ordered by first use.  The sync engine needs ~650ns
            # per DMA_DIRECT2D issue, so the critical prefix (x first half,
            # K/Q weights of pairs 0-3) is issued from three engines in
            # parallel; the rest dribbles out on sync in deadline order. ----
            AH = A * H
            # Per-e-tile DMAs (monolithic multi-tile DMAs run on a single DMA
            # engine and are slower), critical prefix split across the two
            # DMA-capable engine queues (sync + scalar, ~160 GB/s each).
            nc.sync.dma_start(ident[:], id_d[:])
            nc.sync.dma_start(msk[:], msk_d[:])
            for t in range(ET):                       # x first 512 c's
                nc.sync.dma_start(xT[:, t, 0:512], xt_d[t * P:(t + 1) * P, 0:512])
            for t in range(ET):                       # K weights, pairs 0-3
                nc.scalar.dma_start(wkqv[:, t, 0:512],
                                    wkqv_d[t * P:(t + 1) * P, 0:512])
            for t in range(ET):                       # Q weights, pairs 0-3
                eng = nc.scalar if t < 4 else nc.sync
                eng.dma_start(wkqv[:, t, AH:AH + 512],
                              wkqv_d[t * P:(t + 1) * P, AH:AH + 512])
            for t in range(ET):                       # x second half
                nc.sync.dma_start(xT[:, t, 512:S], xt_d[t * P:(t + 1) * P, 512:S])
            for t in range(ET):                       # V weights, f 0:512
                nc.sync.dma_start(wkqv[:, t, 2 * AH:2 * AH + 512],
                                  wkqv_d[t * P:(t + 1) * P, 2 * AH:2 * AH + 512])
            for t in range(ET):                       # K weights, pairs 4-7
                nc.sync.dma_start(wkqv[:, t, 512:AH],
                                  wkqv_d[t * P:(t + 1) * P, 512:AH])
            for t in range(ET):                       # Q weights, pairs 4-7
                nc.sync.dma_start(wkqv[:, t, AH + 512:2 * AH],
                                  wkqv_d[t * P:(t + 1) * P, AH + 512:2 * AH])
            for t in range(ET):                       # V weights, f 512:1024
                nc.sync.dma_start(wkqv[:, t, 2 * AH + 512:3 * AH],
                                  wkqv_d[t * P:(t + 1) * P, 2 * AH + 512:3 * AH])
            for t in range(ET):                       # output projection weights
                nc.sync.dma_start(wo[:, t, :], wo_d[t * P:(t + 1) * P, :])

            K_T = kqv.tile([P, A // 2, S], f16, tag="K_T")   # pair-stacked [2h, c]
            Q_T = kqv.tile([P, A // 2, S], f16, tag="Q_T")
            V = kqv.tile([P, CT, A * H], f16, tag="V")       # [c, f]
            zT = kqv.tile([P, A // 2, S], f16, tag="zT")     # pair-stacked [f, C]
            oacc = kqv.tile([P, CT, E], f16, tag="oacc")     # out_A parking

            # ---- PE warm-up: ride out the HAM throttle during the DMA wait ----
            wps = psq.tile([P, 512], f32, tag="psq", name="warm")
            for w in range(28):
                nc.tensor.matmul(wps[:], ident[:], warm[:],
                                 start=(w == 0), stop=(w == 27),
                                 skip_group_check=True)
            wsb = stats.tile([P, 1], f32, tag="wsink", name="warmsink")
            nc.vector.reduce_max(wsb[:], wps[:, :P], axis=mybir.AxisListType.X)

            # ---- dense projection groups (the filler pool) ----
            def kq_span(p, mat, lo, hi):
                dst = K_T if mat == 0 else Q_T
                w = hi - lo
                ps = psq.tile([P, 512], f32, tag="psq", name=f"q{p}{mat}{lo}")
                for et in range(ET):
                    nc.tensor.matmul(
                        ps[:, 0:w],
                        wkqv[:, et, mat * A * H + p * P: mat * A * H + (p + 1) * P],
                        xT[:, et, lo:hi],
                        start=(et == 0), stop=(et == ET - 1),
                    )
                nc.vector.tensor_copy(out=dst[:, p, lo:hi], in_=ps[:, 0:w])

            def kq_group(p, mat, cc):
                kq_span(p, mat, cc * 512, (cc + 1) * 512)

            def v_group(fc, i):
                ps = psq.tile([P, 512], f32, tag="psq", name=f"v{fc}{i}")
                for et in range(ET):
                    nc.tensor.matmul(
                        ps[:],
                        xT[:, et, i * P:(i + 1) * P],
                        wkqv[:, et, 2 * A * H + fc * 512: 2 * A * H + (fc + 1) * 512],
                        start=(et == 0), stop=(et == ET - 1),
                    )
                nc.vector.tensor_copy(out=V[:, i, fc * 512:(fc + 1) * 512], in_=ps[:])

            def oa_group(m, n_):
                """out_A: first-half contraction (zT pairs 0-3), parked fp16."""
                ps = psq.tile([P, 512], f32, tag="psq", name=f"oa{m}{n_}")
                for p2 in range(4):
                    nc.tensor.matmul(
                        ps[:],
                        zT[:, p2, m * P:(m + 1) * P],
                        wo[:, p2, n_ * 512:(n_ + 1) * 512],
                        start=(p2 == 0), stop=(p2 == 3),
                    )
                nc.vector.tensor_copy(out=oacc[:, m, n_ * 512:(n_ + 1) * 512], in_=ps[:])

            # ---- attention ----
            def attn_rows(p, pump, row_done=None):
                """Scores+exp rows of pair p.  pump(i) interleaves filler work
                after row i's exp chain; row_done(i, ...) optionally self-hosts
                AV steps (pair 7)."""
                heads = [(2 * p, 0), (2 * p + 1, H)]
                Ets = [epool.tile([P, EW], f16, tag="E", name=f"E{k}_{p}")
                       for k in range(2)]
                scat = stats.tile([P, 16], f32, tag="ssum", name=f"sc{p}")
                for i in range(CT):
                    vw = (i + 1) * P          # causally-valid row width
                    if i < 4:   # both heads side by side in one 2-bank tile
                        row = pss.tile([P, 1024], f32, tag="srow", name=f"r_{p}_{i}")
                        views = [row[:, 0:vw], row[:, 512:512 + vw]]
                        dviews = [row[:, i * P:(i + 1) * P],
                                  row[:, 512 + i * P:512 + i * P + P]]
                        n_i = 1
                    else:
                        rows = [pss.tile([P, 1024], f32, tag="srow",
                                         name=f"r{k}_{p}_{i}")
                                for k in range(2)]
                        views = [rows[0][:, 0:vw], rows[1][:, 0:vw]]
                        dviews = [rows[0][:, i * P:(i + 1) * P],
                                  rows[1][:, i * P:(i + 1) * P]]
                        n_i = 2
                    for j in range(n_i):
                        diag = j == n_i - 1
                        ntrim = vw - j * 512 if diag else 512
                        for k, (a, off) in enumerate(heads):
                            nc.tensor.matmul(
                                views[k][:, j * 512:j * 512 + ntrim],
                                K_T[off:off + H, p, i * P:(i + 1) * P],
                                Q_T[off:off + H, p, j * 512:j * 512 + ntrim],
                                start=True, stop=not diag,
                                skip_group_check=True,
                            )
                        if diag:
                            # causal mask of the diagonal block via identity
                            # matmul — keeps the scores->exp chain on the PE
                            # (a DVE tensor_tensor here serializes every row
                            # behind queued casts: +25us measured)
                            for k in range(2):
                                nc.tensor.matmul(
                                    dviews[k],
                                    ident[:], msk[:],
                                    start=False, stop=True,
                                    skip_group_check=True,
                                )
                    for k, (a, off) in enumerate(heads):
                        nc.scalar.activation(
                            Ets[k][:, _off(i):_off(i) + vw], views[k], Exp,
                            accum_out=scat[:, k * 8 + i:k * 8 + i + 1],
                        )
                    pump(i)
                    if row_done is not None:
                        row_done(i, heads, Ets, scat)
                return heads, Ets, scat

            def av_step(p, heads, Ets, state, i):
                """One ascending AV accumulation step (both 512-chunks)."""
                if i == 0:
                    state[0] = pza.tile([P, 512], f32, tag="za", name=f"za_{p}_0")
                if i == 4:
                    state[1] = pza.tile([P, 512], f32, tag="za", name=f"za_{p}_1")
                for j in range(NC):
                    if i < 4 * j:
                        continue
                    ntrim = min(512, (i + 1) * P - j * 512)
                    for k, (a, off) in enumerate(heads):
                        nc.tensor.matmul(
                            state[j][off:off + H, :ntrim],
                            V[:, i, a * H:(a + 1) * H],
                            Ets[k][:, _off(i) + j * 512:_off(i) + j * 512 + ntrim],
                            start=(i == 4 * j), stop=(i == CT - 1),
                            skip_group_check=True,
                        )

            def scale_all(p, heads, scat):
                """Fold 1/rowsum into V rows of pair p (all 8 row-tiles)."""
                rcp = stats.tile([P, 16], f32, tag="rcp", name=f"rc{p}")
                nc.vector.reciprocal(rcp[:], scat[:])
                for k, (a, off) in enumerate(heads):
                    vs = V[:, :, a * H:(a + 1) * H]
                    nc.vector.tensor_tensor(
                        vs, vs,
                        rcp[:, k * 8:(k + 1) * 8, None].to_broadcast([P, CT, H]),
                        mybir.AluOpType.mult,
                    )

            def av_closures(p, heads, Ets, scat):
                """Lazy normalize + AV of pair p, interleaved into pair p+1.
                Pair 0's chain is pushed to row 3+ so its V groups (gated on
                the V-weight DMA) get breathing room."""
                off = 3 if p == 0 else 0
                state = {}
                cs = [(off, lambda: scale_all(p, heads, scat))]
                for i in range(CT):
                    cs.append((min(7, off + (i + 1) // 2),
                               lambda i=i: av_step(p, heads, Ets, state, i)))
                cs.append((min(7, off + 4), lambda: nc.vector.tensor_copy(
                    out=zT[:, p, 0:512], in_=state[0][:])))
                cs.append((min(7, off + 5), lambda: nc.vector.tensor_copy(
                    out=zT[:, p, 512:1024], in_=state[1][:])))
                return cs

            # ---- filler schedule -------------------------------------------
            # Queue of (deadline (pair,row), closure); before row r of pair p
            # every filler with deadline <= (p, r) is forced out.
            fillers = []

            def add(dl, cl):
                fillers.append((dl, cl))

            # pair 0/1 KQ halves not covered by the (minimal) pre-phase
            add((0, 0), lambda: kq_group(1, 0, 0))
            add((0, 1), lambda: kq_group(1, 1, 0))
            add((0, 2), lambda: kq_group(0, 0, 1))
            add((0, 3), lambda: kq_group(0, 1, 1))
            add((0, 5), lambda: kq_group(1, 0, 1))
            add((0, 6), lambda: kq_group(1, 1, 1))
            for p in range(2, 8):
                add((p - 2, 4), lambda p=p: kq_group(p, 0, 0))
                add((p - 2, 6), lambda p=p: kq_group(p, 1, 0))
                add((p - 1, 1), lambda p=p: kq_group(p, 0, 1))
                add((p - 1, 2), lambda p=p: kq_group(p, 1, 1))
            # V(fc0, i) needed by scale_all(0) at pair-1 row 3;
            # V(fc1, i) by scale_all(4) at pair-5 row 0.
            v0_dl = [(0, 4), (0, 6), (0, 7), (1, 0), (1, 0), (1, 1), (1, 2), (1, 3)]
            for i in range(CT):
                add(v0_dl[i], lambda i=i: v_group(0, i))
            v1_dl = [(2, 1), (2, 3), (2, 5), (3, 1), (3, 3), (4, 1), (4, 3), (4, 5)]
            for i in range(CT):
                add(v1_dl[i], lambda i=i: v_group(1, i))
            # out_A(m, n) after AV(3) copies (pair-4 row 5); spread pairs 4-7.
            oa_dl = [(4, 6), (4, 7), (5, 1), (5, 3), (5, 5), (5, 7),
                     (6, 1), (6, 2), (6, 3), (6, 4), (6, 5), (6, 6),
                     (7, 1), (7, 2), (7, 3), (7, 4)]
            for m in range(CT):
                for n_ in range(NC):
                    add(oa_dl[2 * m + n_], lambda m=m, n_=n_: oa_group(m, n_))

            fillers.sort(key=lambda x: x[0])
            fidx = [0]
            av_queue = []   # (deadline_row, closure) of the previous pair's AV

            def pump_factory(p):
                def pump(i):
                    while fidx[0] < len(fillers) and fillers[fidx[0]][0] <= (p, i):
                        fillers[fidx[0]][1]()
                        fidx[0] += 1
                    while av_queue and av_queue[0][0] <= i:
                        av_queue.pop(0)[1]()
                return pump

            # pre-phase: just KQ(0) first half — rows 0-3 of pair 0 need
            # nothing else, so attention starts as early as possible.
            kq_group(0, 0, 0)
            kq_group(0, 1, 0)

            for p in range(8):
                if p == 7:
                    # Self-host AV(7), but only from row 5 on: the za PSUM ring
                    # slots are still owned by AV(6) (drained via the pump at
                    # rows <= 5); claiming them earlier would head-of-line
                    # block the PE queue on AV(6)'s own not-yet-issued matmuls.
                    st7 = {}

                    def scale7(ii, heads, scat):
                        rcp2 = stats.tile([P, 2], f32, tag="rcp2", name=f"r7_{ii}")
                        for k in range(2):
                            nc.vector.reciprocal(
                                rcp2[:, k:k + 1],
                                scat[:, k * 8 + ii:k * 8 + ii + 1])
                        for k, (a, off) in enumerate(heads):
                            nc.vector.tensor_scalar_mul(
                                V[:, ii, a * H:(a + 1) * H],
                                V[:, ii, a * H:(a + 1) * H],
                                rcp2[:, k:k + 1],
                            )

                    def row7_done(i, heads, Ets, scat):
                        if i < 5:
                            return
                        first = range(6) if i == 5 else [i]
                        for ii in first:
                            scale7(ii, heads, scat)
                            av_step(7, heads, Ets, st7, ii)

                    heads, Ets, scat = attn_rows(p, pump_factory(p), row7_done)
                    nc.vector.tensor_copy(out=zT[:, 7, 0:512], in_=st7[0][:])
                    nc.vector.tensor_copy(out=zT[:, 7, 512:1024], in_=st7[1][:])
                else:
                    heads, Ets, scat = attn_rows(p, pump_factory(p))
                    av_queue = av_closures(p, heads, Ets, scat)
            while fidx[0] < len(fillers):
                fillers[fidx[0]][1]()
                fidx[0] += 1

            # ---- output projection, second half + combine ----
            # One 2-bank PSUM tile per m-tile (from the now-idle score pool),
            # a single wide tensor_tensor add against the parked first half,
            # and a single output DMA — the tail is PE-paced, not DVE/DMA-
            # issue-paced.
            for m in range(CT):
                ps = pss.tile([P, 1024], f32, tag="srow", name=f"ob{m}")
                for n_ in range(NC):
                    for p2 in range(4, ET):
                        nc.tensor.matmul(
                            ps[:, n_ * 512:(n_ + 1) * 512],
                            zT[:, p2, m * P:(m + 1) * P],
                            wo[:, p2, n_ * 512:(n_ + 1) * 512],
                            start=(p2 == 4), stop=(p2 == ET - 1),
                        )
                ot = outp.tile([P, 1024], f16, tag="ot")
                nc.vector.tensor_tensor(
                    ot[:], ps[:], oacc[:, m, :], mybir.AluOpType.add,
                )
                nc.sync.dma_start(out_d[m * P:(m + 1) * P, :], ot[:])

    # HW allows only one sync-wait per instruction (matmuls especially);
    # split excess waits into InstEventSemaphore like the bacc layer does.
    import bass_rust
    bass_rust.generate_event_semaphores(nc)
    return nc


def _host_prep(x, W_K, W_Q, W_V, W_O):
    """Pack per-core input dicts (host-side layout prep, fp16 casts)."""
    wk = W_K.transpose(2, 0, 1).reshape(E, A * H)
    wq = (W_Q / np.sqrt(H)).transpose(2, 0, 1).reshape(E, A * H)
    wv = W_V.transpose(2, 0, 1).reshape(E, A * H)
    wkqv = np.concatenate([wk, wq, wv], axis=1).astype(np.float16)
    wo = np.ascontiguousarray(W_O.T).astype(np.float16)

    r = np.arange(P)[:, None]
    d = np.arange(P)[None, :]
    msk = np.where(d <= r, 0.0, NEG).astype(np.float16)   # causal 128-block
    ident = np.eye(P, dtype=np.float16)

    in_maps = []
    for b in range(B):
        in_maps.append({
            "xt": np.ascontiguousarray(x[b].T).astype(np.float16),
            "wkqv": wkqv,
            "wo": wo,
            "msk": msk,
            "ident": ident,
        })
    return in_maps


def _run(x, W_K, W_Q, W_V, W_O, **spmd_kwargs):
    from concourse.bass_utils import run_bass_kernel_spmd

    if "nc" not in _cache:
        _cache["nc"] = _build_nc()
    in_maps = _host_prep(
        np.asarray(x, dtype=np.float32), np.asarray(W_K, dtype=np.float32),
        np.asarray(W_Q, dtype=np.float32), np.asarray(W_V, dtype=np.float32),
        np.asarray(W_O, dtype=np.float32),
    )
    res = run_bass_kernel_spmd(_cache["nc"], in_maps, core_ids=list(range(B)),
                               **spmd_kwargs)
    out = np.stack([r["out"] for r in res.results], axis=0).astype(np.float32)
    return out, res


def kernel(x, W_K, W_Q, W_V, W_O):
    out, _ = _run(x, W_K, W_Q, W_V, W_O)
    return out
